# revision 10
# baseline (speedup 1.0000x reference)
"""Trainium2 Bass kernel for the DLEDMD problem.

Contract: kernel(**inputs) takes the FULL unsharded inputs (numpy arrays,
keyed as in setup_inputs()) and returns the FULL output tuple matching
reference():  (y, x_ae, x_adv, y_adv_real, y_adv_imag, k_evals, k_efuns,
k_modes).

Strategy
--------
 * Pure data parallel across 8 NeuronCores: batch 128 -> 16 per core.
 * Device computes the two heavy MLPs (encoder y = mlp(x), decoder
   x_ae = mlp(y)); activations kept TRANSPOSED [features, tokens] on chip
   so no on-device transposes are needed (host transposes in/out).
 * The LAPACK-convention-bound EDMD chain (SVD -> pinv -> eig -> solve ->
   scan) and the tiny [128,4,64] decoder pass run on host in float64,
   reproducing the reference bit-for-bit up to BLAS rounding.
 * Matmul precision scheme (SCHEME):
     - "fp32": native float32 matmuls (4 cycles/row on PE).
     - "r1":   float32r single pass (1 cycle/row, ~11 mantissa bits).
     - "r3":   float32r hi/lo split, 3 passes -> ~fp32 accuracy at 3 cycles.
"""

import os
import sys

import numpy as np

try:
    import concourse.bass as bass  # noqa: F401
except Exception:  # pragma: no cover
    sys.path.insert(0, "/opt/trn_rl_repo")

import concourse.bacc as bacc
import concourse.tile as tile
from concourse import mybir
from concourse.bass_utils import run_bass_kernel_spmd

# ---------------------------------------------------------------- constants
B, T, D, L, H = 128, 256, 64, 64, 256
P_STEPS, DT_STEP = 4, 1.0
N_CORES = 8
B_SH = B // N_CORES            # 16 batches per core
NTOK = B_SH * T                # 4096 tokens per core
NW = 512                       # tokens per PSUM chunk
N_CH = NTOK // NW              # 8 chunks

SCHEME = os.environ.get("DLEDMD_SCHEME", "r3")

# layer table: (name, fin, fout, relu)
ENC = [("e0", D, H, True), ("e1", H, H, True), ("e2", H, H, True), ("e3", H, L, False)]
DEC = [("d0", L, H, True), ("d1", H, H, True), ("d2", H, H, True), ("d3", H, D, False)]

_HI_MASK = np.uint32(0xFFFFF000)  # keep 11 explicit mantissa bits


def _chunks(n):
    """split feature dim n into partition chunks of <=128"""
    out = []
    s = 0
    while s < n:
        c = min(128, n - s)
        out.append((s, c))
        s += c
    return out


def _split_hi_lo(a):
    """split fp32 array into (hi, lo) with 11-bit-mantissa hi via truncation"""
    a = np.ascontiguousarray(a, dtype=np.float32)
    hi = (a.view(np.uint32) & _HI_MASK).view(np.float32)
    lo = ((a - hi).view(np.uint32) & _HI_MASK).view(np.float32)
    return hi, np.ascontiguousarray(lo)


def _build_nc(scheme, repeat=1):
    f32 = mybir.dt.float32
    f32r = mybir.dt.float32r
    mm_dt = f32 if scheme == "fp32" else f32r
    act_t = mybir.ActivationFunctionType

    nc = bacc.Bacc("TRN2", target_bir_lowering=False, debug=False)

    # ---------------- DRAM I/O ----------------
    # activations input (transposed) and weights; r3 passes hi/lo pre-split
    def din(name, shape, dt):
        return nc.dram_tensor(name, shape, dt, kind="ExternalInput").ap()

    if scheme == "r3":
        x_in = [din("xT_hi", [D, NTOK], mm_dt), din("xT_lo", [D, NTOK], mm_dt)]
    else:
        x_in = [din("xT", [D, NTOK], mm_dt)]

    w_in = {}
    b_in = {}
    for lname, fin, fout, _ in ENC + DEC:
        if scheme == "r3":
            w_in[lname] = [din(f"w_{lname}_hi", [fin, fout], mm_dt),
                           din(f"w_{lname}_lo", [fin, fout], mm_dt)]
        else:
            w_in[lname] = [din(f"w_{lname}", [fin, fout], mm_dt)]
        b_in[lname] = din(f"b_{lname}", [fout, 1], f32)

    y_out = nc.dram_tensor("yT", [L, NTOK], f32, kind="ExternalOutput").ap()
    xae_out = nc.dram_tensor("xaeT", [D, NTOK], f32, kind="ExternalOutput").ap()

    with tile.TileContext(nc) as tc:
        with (
            tc.tile_pool(name="wpool", bufs=1) as wpool,
            tc.tile_pool(name="hi", bufs=4) as hi_pool,
            tc.tile_pool(name="lo", bufs=4) as lo_pool,
            tc.tile_pool(name="full", bufs=6) as full_pool,
            tc.tile_pool(name="outb", bufs=6) as out_pool,
            tc.tile_pool(name="ps", bufs=8, space="PSUM") as ps_pool,
        ):
            # ---------- load weights + biases (once, outside any repeat) ----
            w_sb = {}   # w_sb[lname][pass][ki] -> tile [ksz, fout]
            b_sb = {}   # b_sb[lname][mi] -> tile [msz, 1]
            for lname, fin, fout, _ in ENC + DEC:
                passes = []
                for wdram in w_in[lname]:
                    ktiles = []
                    for (ks, ksz) in _chunks(fin):
                        wt = wpool.tile([ksz, fout], mm_dt,
                                        tag=f"w_{lname}_{ks}_{len(passes)}",
                                        name=f"w_{lname}_{ks}_{len(passes)}")
                        nc.sync.dma_start(out=wt, in_=wdram[ks:ks + ksz, :])
                        ktiles.append(wt)
                    passes.append(ktiles)
                w_sb[lname] = passes
                bts = []
                for (ms, msz) in _chunks(fout):
                    bt = wpool.tile([msz, 1], f32, tag=f"b_{lname}_{ms}",
                                    name=f"b_{lname}_{ms}")
                    nc.sync.dma_start(out=bt, in_=b_in[lname][ms:ms + msz, :])
                    bts.append(bt)
                b_sb[lname] = bts

            def body():
                # ---- load x (generation-0 activations) ----
                def load_xt(dram, pool, tag, name):
                    t = pool.tile([D, NTOK], mm_dt, tag=tag, name=name)
                    for n in range(N_CH):
                        nc.sync.dma_start(out=t[:, n * NW:(n + 1) * NW],
                                          in_=dram[:, n * NW:(n + 1) * NW])
                    return t

                cur_hi = [(D, load_xt(x_in[0], hi_pool, "hi", "x_hi"))]
                cur_lo = ([(D, load_xt(x_in[1], lo_pool, "lo", "x_lo"))]
                          if scheme == "r3" else None)

                evac_flip = 0  # alternate ACT / DVE for relu evacuation
                for li, (lname, fin, fout, relu) in enumerate(ENC + DEC):
                    is_enc_out = lname == "e3"   # y, needed as decoder input
                    is_final = not relu
                    mchunks = _chunks(fout)
                    kchunks = _chunks(fin)
                    # matmul passes: r3 (w_hi,a_hi),(w_hi,a_lo),(w_lo,a_hi)
                    if scheme == "r3":
                        pair_sel = [(0, cur_hi), (0, cur_lo), (1, cur_hi)]
                    else:
                        pair_sel = [(0, cur_hi)]

                    new_hi = []
                    new_lo = []
                    if (not is_final) or is_enc_out:
                        for (ms, msz) in mchunks:
                            new_hi.append((msz, hi_pool.tile(
                                [msz, NTOK], mm_dt, tag="hi", name=f"h_{lname}_{ms}")))
                            if scheme == "r3":
                                new_lo.append((msz, lo_pool.tile(
                                    [msz, NTOK], mm_dt, tag="lo", name=f"hlo_{lname}_{ms}")))

                    for mi, (ms, msz) in enumerate(mchunks):
                        bias = b_sb[lname][mi]
                        for n in range(N_CH):
                            nsl = slice(n * NW, (n + 1) * NW)
                            ps = ps_pool.tile([msz, NW], f32, tag="ps",
                                              name=f"ps_{lname}_{ms}_{n}")
                            mms = []
                            for (wp, acts) in pair_sel:
                                for ki, (ks, ksz) in enumerate(kchunks):
                                    mms.append((w_sb[lname][wp][ki][:, ms:ms + msz],
                                                acts[ki][1][:, nsl]))
                            for i, (lhsT, rhs) in enumerate(mms):
                                nc.tensor.matmul(ps, lhsT, rhs,
                                                 start=(i == 0), stop=(i == len(mms) - 1))
                            # ----- evacuate PSUM -----
                            if not is_final:
                                if scheme == "r3":
                                    fullt = full_pool.tile(
                                        [msz, NW], f32, tag="full",
                                        name=f"full_{lname}_{ms}_{n}")
                                    nc.scalar.activation(fullt, ps, act_t.Relu, bias=bias)
                                    # hi = round_to_f32r(full) (DVE write-rounds)
                                    nc.vector.tensor_copy(new_hi[mi][1][:, nsl], fullt)
                                    # lo = full - hi (f32r write-round harmless)
                                    nc.vector.tensor_tensor(
                                        out=new_lo[mi][1][:, nsl], in0=fullt,
                                        in1=new_hi[mi][1][:, nsl].bitcast(f32),
                                        op=mybir.AluOpType.subtract)
                                else:
                                    dst = new_hi[mi][1][:, nsl]
                                    if evac_flip % 2 == 0:
                                        nc.scalar.activation(dst, ps, act_t.Relu, bias=bias)
                                    else:
                                        nc.vector.tensor_scalar(
                                            out=dst, in0=ps, scalar1=bias, scalar2=0.0,
                                            op0=mybir.AluOpType.add,
                                            op1=mybir.AluOpType.max)
                                    evac_flip += 1
                            else:
                                outt = out_pool.tile([msz, NW], f32, tag="outb",
                                                     name=f"out_{lname}_{ms}_{n}")
                                nc.scalar.activation(outt, ps, act_t.Identity, bias=bias)
                                dram = y_out if is_enc_out else xae_out
                                nc.sync.dma_start(out=dram[ms:ms + msz, nsl], in_=outt)
                                if is_enc_out:
                                    # decoder consumes y as f32r (+ lo for r3)
                                    nc.vector.tensor_copy(new_hi[mi][1][:, nsl], outt)
                                    if scheme == "r3":
                                        nc.vector.tensor_tensor(
                                            out=new_lo[mi][1][:, nsl], in0=outt,
                                            in1=new_hi[mi][1][:, nsl].bitcast(f32),
                                            op=mybir.AluOpType.subtract)

                    if (not is_final) or is_enc_out:
                        cur_hi = new_hi
                        cur_lo = new_lo if scheme == "r3" else None

            # Always wrap in a hardware loop (even for repeat=1): walrus's
            # embedded birsim skips dynamic loops, cutting NEFF compile time
            # from ~6 min to ~3 s for the flat body at ~us runtime cost.
            with tc.For_i(0, repeat, 1):
                body()

    nc.finalize()
    return nc


_NC_CACHE = {}


def _get_nc(scheme):
    if scheme not in _NC_CACHE:
        _NC_CACHE[scheme] = _build_nc(scheme)
    return _NC_CACHE[scheme]


# ------------------------------------------------------------- host helpers
def _host_chain(x, ews, ebs, dws, dbs, in_dtype):
    """Bit-exact replication of the reference EDMD chain on CPU via eager jax.

    The LAPACK eigenvector phase convention (largest component real) is
    discontinuous in the input bits, so the Koopman operator K must match the
    reference bit-for-bit.  Running the identical op sequence eagerly through
    jax-on-CPU achieves that.
    Returns numpy arrays: (y_host, k_efuns, k_evals, k_modes, y_adv, x_adv).
    """
    import jax
    import jax.numpy as jnp

    jax.config.update("jax_enable_x64", True)
    cpu = jax.devices("cpu")[0]

    def mlp(h, ws, bs):
        for i, (w, b) in enumerate(zip(ws, bs)):
            h = h @ w + b
            if i < len(ws) - 1:
                h = jax.nn.relu(h)
        return h

    with jax.default_device(cpu):
        put = lambda a: jax.device_put(np.asarray(a, dtype=in_dtype), cpu)
        xj = put(x)
        ewsj = [put(w) for w in ews]
        ebsj = [put(b) for b in ebs]
        dwsj = [put(w) for w in dws]
        dbsj = [put(b) for b in dbs]

        y = mlp(xj, ewsj, ebsj)                 # [B, T, L]
        yt = jnp.swapaxes(y, 1, 2)              # [B, L, T]
        xt = jnp.swapaxes(xj, 1, 2)             # [B, D, T]

        y_m, y_p = yt[:, :, :-1], yt[:, :, 1:]
        U, S, Vh = jnp.linalg.svd(y_m, full_matrices=False)
        V = jnp.swapaxes(Vh, -1, -2)
        pinv = (V / S[:, None, :]) @ jnp.swapaxes(U, -1, -2)
        Kop = y_p @ pinv
        evals, modes = jnp.linalg.eig(Kop)
        k_evals = jnp.log(evals) / DT_STEP
        k_efuns = jnp.linalg.solve(modes, xt.astype(modes.dtype))
        xint = k_efuns[:, :, -1:]

        def step(e, _):
            yp = (modes @ (e[:, :, None] * xint))[..., 0]
            return e * e, yp

        _, ys = jax.lax.scan(step, k_evals * k_evals, None, length=P_STEPS)
        y_adv = jnp.transpose(ys, (1, 0, 2))    # [B, P, L] complex
        y_adv_real = jnp.real(y_adv)
        x_adv = mlp(y_adv_real, dwsj, dbsj)     # [B, P, D]

    return (np.asarray(y), np.asarray(k_efuns), np.asarray(k_evals),
            np.asarray(modes), np.asarray(y_adv), np.asarray(x_adv))


def _in_maps_for(scheme, x, ws, bs):
    """Build the per-core input maps."""
    maps = []
    w_np = {}
    for (lname, fin, fout, _), w in zip(ENC + DEC, ws):
        w32 = np.ascontiguousarray(w, dtype=np.float32)
        if scheme == "r3":
            hi, lo = _split_hi_lo(w32)
            w_np[f"w_{lname}_hi"] = hi
            w_np[f"w_{lname}_lo"] = lo
        else:
            w_np[f"w_{lname}"] = w32
    b_np = {}
    for (lname, fin, fout, _), b in zip(ENC + DEC, bs):
        b_np[f"b_{lname}"] = np.ascontiguousarray(b, dtype=np.float32).reshape(-1, 1)

    for c in range(N_CORES):
        xs = x[c * B_SH:(c + 1) * B_SH].reshape(NTOK, D)
        xT = np.ascontiguousarray(xs.T, dtype=np.float32)
        m = {}
        if scheme == "r3":
            hi, lo = _split_hi_lo(xT)
            m["xT_hi"] = hi
            m["xT_lo"] = lo
        else:
            m["xT"] = xT
        m.update(w_np)
        m.update(b_np)
        maps.append(m)
    return maps


def kernel(**inputs):
    x = np.asarray(inputs["x"])
    in_dtype = x.dtype if x.dtype in (np.float32, np.float64) else np.float64
    cdtype = np.complex64 if in_dtype == np.float32 else np.complex128

    ews = [np.asarray(inputs[f"enc_w{i}"]) for i in range(4)]
    ebs = [np.asarray(inputs[f"enc_b{i}"]) for i in range(4)]
    dws = [np.asarray(inputs[f"dec_w{i}"]) for i in range(4)]
    dbs = [np.asarray(inputs[f"dec_b{i}"]) for i in range(4)]

    # ---------------- device: y = enc(x), x_ae = dec(y) ----------------
    scheme = SCHEME
    nc = _get_nc(scheme)
    in_maps = _in_maps_for(scheme, x.astype(np.float32, copy=False), ews + dws, ebs + dbs)
    res = run_bass_kernel_spmd(nc, in_maps, core_ids=list(range(N_CORES)))

    y_dev = np.empty((B, T, L), dtype=np.float32)
    xae_dev = np.empty((B, T, D), dtype=np.float32)
    for c in range(N_CORES):
        y_dev[c * B_SH:(c + 1) * B_SH] = res.results[c]["yT"].T.reshape(B_SH, T, L)
        xae_dev[c * B_SH:(c + 1) * B_SH] = res.results[c]["xaeT"].T.reshape(B_SH, T, D)

    # ------- host: bit-exact reference EDMD chain in input precision -------
    (_y_host, k_efuns, k_evals, k_modes, y_adv, x_adv) = _host_chain(
        x, ews, ebs, dws, dbs, in_dtype)
    y_adv_real = np.real(y_adv)
    y_adv_imag = np.imag(y_adv)

    return (
        y_dev.astype(in_dtype),
        xae_dev.astype(in_dtype),
        x_adv.astype(in_dtype),
        y_adv_real.astype(in_dtype),
        y_adv_imag.astype(in_dtype),
        k_evals.astype(cdtype),
        k_efuns.astype(cdtype),
        k_modes.astype(cdtype),
    )


# revision 25
# speedup vs baseline: 1.0074x; 1.0074x over previous
"""Trainium2 Bass kernel for the DLEDMD problem.

Contract: kernel(**inputs) takes the FULL unsharded inputs (numpy arrays,
keyed as in setup_inputs()) and returns the FULL output tuple matching
reference():  (y, x_ae, x_adv, y_adv_real, y_adv_imag, k_evals, k_efuns,
k_modes).

Strategy
--------
 * Pure data parallel across 8 NeuronCores: batch 128 -> 16 per core.
 * Device computes the two heavy MLPs (encoder y = mlp(x), decoder
   x_ae = mlp(y)); activations kept TRANSPOSED [features, tokens] on chip
   so no on-device transposes are needed (host transposes in/out).
 * The LAPACK-convention-bound EDMD chain (SVD -> pinv -> eig -> solve ->
   scan) and the tiny [128,4,64] decoder pass run on host in float64,
   reproducing the reference bit-for-bit up to BLAS rounding.
 * Matmul precision scheme (SCHEME):
     - "fp32": native float32 matmuls (4 cycles/row on PE).
     - "r1":   float32r single pass (1 cycle/row, ~11 mantissa bits).
     - "r3":   float32r hi/lo split, 3 passes -> ~fp32 accuracy at 3 cycles.
"""

import os
import sys

import numpy as np

try:
    import concourse.bass as bass  # noqa: F401
except Exception:  # pragma: no cover
    sys.path.insert(0, "/opt/trn_rl_repo")

import concourse.bacc as bacc
import concourse.tile as tile
from concourse import mybir
from concourse.bass_utils import run_bass_kernel_spmd

# ---------------------------------------------------------------- constants
B, T, D, L, H = 128, 256, 64, 64, 256
P_STEPS, DT_STEP = 4, 1.0
N_CORES = 8
B_SH = B // N_CORES            # 16 batches per core
NTOK = B_SH * T                # 4096 tokens per core
NW = 512                       # tokens per PSUM chunk
N_CH = NTOK // NW              # 8 chunks

SCHEME = os.environ.get("DLEDMD_SCHEME", "r3")

# layer table: (name, fin, fout, relu)
ENC = [("e0", D, H, True), ("e1", H, H, True), ("e2", H, H, True), ("e3", H, L, False)]
DEC = [("d0", L, H, True), ("d1", H, H, True), ("d2", H, H, True), ("d3", H, D, False)]

_HI_MASK = np.uint32(0xFFFFF000)  # keep 11 explicit mantissa bits


def _chunks(n):
    """split feature dim n into partition chunks of <=128"""
    out = []
    s = 0
    while s < n:
        c = min(128, n - s)
        out.append((s, c))
        s += c
    return out


def _split_hi_lo(a):
    """split fp32 array into (hi, lo) with 11-bit-mantissa hi via truncation"""
    a = np.ascontiguousarray(a, dtype=np.float32)
    hi = (a.view(np.uint32) & _HI_MASK).view(np.float32)
    lo = ((a - hi).view(np.uint32) & _HI_MASK).view(np.float32)
    return hi, np.ascontiguousarray(lo)


def _build_nc(scheme, repeat=1, zero_bias=True, loop=True,
              x_ring="scalar", store_ring="sync", hilo_bufs=4, ps_bufs=8,
              x_dmas=2, staged_store=False):
    f32 = mybir.dt.float32
    f32r = mybir.dt.float32r
    mm_dt = f32 if scheme == "fp32" else f32r
    act_t = mybir.ActivationFunctionType

    nc = bacc.Bacc("TRN2", target_bir_lowering=False, debug=False)

    # ---------------- DRAM I/O ----------------
    # activations input (transposed) and weights; r3 passes hi/lo pre-split
    def din(name, shape, dt):
        return nc.dram_tensor(name, shape, dt, kind="ExternalInput").ap()

    if scheme == "r3":
        # stacked gen-0 input: rows 0..63 = x_hi, rows 64..127 = x_lo; the
        # e0 weights are K-stacked to match ([w_hi;w_hi] and [w_lo;0]), so
        # layer e0 runs 2 full-K=128 passes instead of 3 half-K=64 ones.
        x_in = [din("xT_hilo", [2 * D, NTOK], mm_dt)]
    else:
        x_in = [din("xT", [D, NTOK], mm_dt)]

    w_in = {}
    b_in = {}
    for lname, fin, fout, _ in ENC + DEC:
        kch = _chunks(fin)
        pshape = [kch[0][1], len(kch) * fout]
        if scheme == "r3" and lname == "e0":
            w_in[lname] = [din("w_e0_p1", [2 * D, fout], mm_dt),
                           din("w_e0_p2", [2 * D, fout], mm_dt)]
        elif scheme == "r3":
            w_in[lname] = [din(f"w_{lname}_hi", pshape, mm_dt),
                           din(f"w_{lname}_lo", pshape, mm_dt)]
        else:
            w_in[lname] = [din(f"w_{lname}", pshape, mm_dt)]
        b_in[lname] = din(f"b_{lname}", [fout, 1], f32)

    y_out = nc.dram_tensor("yT", [L, NTOK], f32, kind="ExternalOutput").ap()
    xae_out = nc.dram_tensor("xaeT", [D, NTOK], f32, kind="ExternalOutput").ap()

    with tile.TileContext(nc) as tc:
        with (
            tc.tile_pool(name="wpool", bufs=1) as wpool,
            tc.tile_pool(name="hi", bufs=hilo_bufs) as hi_pool,
            tc.tile_pool(name="lo", bufs=hilo_bufs) as lo_pool,
            tc.tile_pool(name="full", bufs=6) as full_pool,
            tc.tile_pool(name="outb", bufs=1) as out_pool,
            tc.tile_pool(name="ps", bufs=ps_bufs, space="PSUM") as ps_pool,
        ):
            # ---------- load weights + biases (once, outside any repeat) ----
            # weights arrive host-packed as [ksz, nk*fout] (k-chunks along the
            # free dim) so each (layer, pass) is a single large DMA -- the
            # HWDGE ring's ~0.6us fixed cost per dma_start otherwise starves
            # the PE during the first layers.
            w_sb = {}   # w_sb[lname][pass][ki] -> AP [ksz, fout]
            b_sb = {}   # b_sb[lname][mi] -> tile [msz, 1]
            for lname, fin, fout, _ in ENC + DEC:
                passes = []
                kch = _chunks(fin)
                if scheme == "r3" and lname == "e0":
                    kch = [(0, 2 * D)]
                for wdram in w_in[lname]:
                    wt = wpool.tile([kch[0][1], len(kch) * fout], mm_dt,
                                    tag=f"w_{lname}_{len(passes)}",
                                    name=f"w_{lname}_{len(passes)}")
                    nc.sync.dma_start(out=wt, in_=wdram)
                    passes.append([wt[:ksz, ki * fout:(ki + 1) * fout]
                                   for ki, (ks, ksz) in enumerate(kch)])
                w_sb[lname] = passes
                bts = []
                if not zero_bias:
                    for (ms, msz) in _chunks(fout):
                        bt = wpool.tile([msz, 1], f32, tag=f"b_{lname}_{ms}",
                                        name=f"b_{lname}_{ms}")
                        nc.sync.dma_start(out=bt, in_=b_in[lname][ms:ms + msz, :])
                        bts.append(bt)
                b_sb[lname] = bts

            def body():
                # ---- load x (generation-0 activations) ----
                # one dma_start per tensor on the scalar HWDGE ring: each
                # dma_start costs ~0.6us of serialized ring time, so fewer,
                # larger transfers win; the ring split keeps x off the
                # weight stream.
                eng = nc.scalar if x_ring == "scalar" else nc.sync
                xdim = 2 * D if scheme == "r3" else D
                xt = hi_pool.tile([xdim, NTOK], mm_dt, tag="hi", name="x_t")
                xw = NTOK // x_dmas
                for c in range(x_dmas):
                    csl = slice(c * xw, (c + 1) * xw)
                    eng.dma_start(out=xt[:, csl], in_=x_in[0][:, csl])
                cur_hi = [(xdim, xt)]
                cur_lo = [(xdim, xt)] if scheme == "r3" else None

                store_eng = nc.scalar if store_ring == "scalar" else nc.sync
                y_stage = xae_stage = None
                if staged_store:
                    y_stage = out_pool.tile([L, NTOK], f32, tag="y_stage",
                                            name="y_stage")
                    xae_stage = out_pool.tile([D, NTOK], f32, tag="xae_stage",
                                              name="xae_stage")

                evac_flip = 0  # alternate ACT / DVE for relu evacuation
                for li, (lname, fin, fout, relu) in enumerate(ENC + DEC):
                    is_enc_out = lname == "e3"   # y, needed as decoder input
                    is_final = not relu
                    mchunks = _chunks(fout)
                    kchunks = _chunks(fin)
                    # matmul passes: r3 (w_hi,a_hi),(w_hi,a_lo),(w_lo,a_hi);
                    # e0 is K-stacked: both passes consume the same rhs
                    if scheme == "r3" and lname == "e0":
                        pair_sel = [(0, cur_hi), (1, cur_hi)]
                        kchunks = [(0, 2 * D)]
                    elif scheme == "r3":
                        pair_sel = [(0, cur_hi), (0, cur_lo), (1, cur_hi)]
                    else:
                        pair_sel = [(0, cur_hi)]

                    new_hi = []
                    new_lo = []
                    if (not is_final) or is_enc_out:
                        for (ms, msz) in mchunks:
                            new_hi.append((msz, hi_pool.tile(
                                [msz, NTOK], mm_dt, tag="hi", name=f"h_{lname}_{ms}")))
                            if scheme == "r3":
                                new_lo.append((msz, lo_pool.tile(
                                    [msz, NTOK], mm_dt, tag="lo", name=f"hlo_{lname}_{ms}")))

                    for mi, (ms, msz) in enumerate(mchunks):
                        bias = None if zero_bias else b_sb[lname][mi]
                        for n in range(N_CH):
                            nsl = slice(n * NW, (n + 1) * NW)
                            ps = ps_pool.tile([msz, NW], f32, tag="ps",
                                              name=f"ps_{lname}_{ms}_{n}")
                            mms = []
                            for (wp, acts) in pair_sel:
                                for ki, (ks, ksz) in enumerate(kchunks):
                                    mms.append((w_sb[lname][wp][ki][:, ms:ms + msz],
                                                acts[ki][1][:, nsl]))
                            for i, (lhsT, rhs) in enumerate(mms):
                                nc.tensor.matmul(ps, lhsT, rhs,
                                                 start=(i == 0), stop=(i == len(mms) - 1))
                            # ----- evacuate PSUM -----
                            if not is_final:
                                if scheme == "r3" and zero_bias:
                                    # 2-op evac: hi = RN(relu(ps)) on ACT,
                                    # lo = relu(ps) - hi fused on DVE
                                    nc.scalar.activation(
                                        new_hi[mi][1][:, nsl], ps, act_t.Relu)
                                    nc.vector.scalar_tensor_tensor(
                                        out=new_lo[mi][1][:, nsl], in0=ps,
                                        scalar=0.0,
                                        in1=new_hi[mi][1][:, nsl].bitcast(f32),
                                        op0=mybir.AluOpType.max,
                                        op1=mybir.AluOpType.subtract)
                                elif scheme == "r3":
                                    fullt = full_pool.tile(
                                        [msz, NW], f32, tag="full",
                                        name=f"full_{lname}_{ms}_{n}")
                                    nc.scalar.activation(fullt, ps, act_t.Relu, bias=bias)
                                    # hi = round_to_f32r(full) (DVE write-rounds)
                                    nc.vector.tensor_copy(new_hi[mi][1][:, nsl], fullt)
                                    # lo = full - hi (f32r write-round harmless)
                                    nc.vector.tensor_tensor(
                                        out=new_lo[mi][1][:, nsl], in0=fullt,
                                        in1=new_hi[mi][1][:, nsl].bitcast(f32),
                                        op=mybir.AluOpType.subtract)
                                else:
                                    dst = new_hi[mi][1][:, nsl]
                                    bias_arg = 0.0 if bias is None else bias
                                    if evac_flip % 2 == 0:
                                        nc.scalar.activation(dst, ps, act_t.Relu,
                                                             bias=bias_arg)
                                    else:
                                        nc.vector.tensor_scalar(
                                            out=dst, in0=ps, scalar1=bias_arg,
                                            scalar2=0.0,
                                            op0=mybir.AluOpType.add,
                                            op1=mybir.AluOpType.max)
                                    evac_flip += 1
                            else:
                                if staged_store:
                                    stage = y_stage if is_enc_out else xae_stage
                                    outt = stage[ms:ms + msz, nsl]
                                else:
                                    outt = out_pool.tile(
                                        [msz, NW], f32, tag="outb",
                                        name=f"out_{lname}_{ms}_{n}", bufs=6)
                                if zero_bias:
                                    nc.scalar.activation(outt, ps, act_t.Copy)
                                else:
                                    nc.scalar.activation(outt, ps, act_t.Identity,
                                                         bias=bias)
                                if not staged_store:
                                    dram = y_out if is_enc_out else xae_out
                                    store_eng.dma_start(
                                        out=dram[ms:ms + msz, nsl], in_=outt)
                                if is_enc_out:
                                    # decoder consumes y as f32r (+ lo for r3)
                                    src_ap = ps if zero_bias else outt
                                    nc.vector.tensor_copy(new_hi[mi][1][:, nsl], src_ap)
                                    if scheme == "r3":
                                        nc.vector.scalar_tensor_tensor(
                                            out=new_lo[mi][1][:, nsl], in0=src_ap,
                                            scalar=0.0,
                                            in1=new_hi[mi][1][:, nsl].bitcast(f32),
                                            op0=mybir.AluOpType.add,
                                            op1=mybir.AluOpType.subtract)

                    if (not is_final) or is_enc_out:
                        cur_hi = new_hi
                        cur_lo = new_lo if scheme == "r3" else None
                    if staged_store and lname == "e3":
                        store_eng.dma_start(out=y_out, in_=y_stage)
                    elif staged_store and lname == "d3":
                        store_eng.dma_start(out=xae_out, in_=xae_stage)

            # Wrap in a hardware loop (even for repeat=1): walrus's embedded
            # birsim skips dynamic loops, cutting NEFF compile time from
            # ~6 min to ~3 s for the flat body at ~us runtime cost.
            if loop:
                with tc.For_i(0, repeat, 1):
                    body()
            else:
                body()

    nc.finalize()
    return nc


_NC_CACHE = {}


def _get_nc(scheme, zero_bias=True):
    key = (scheme, zero_bias)
    if key not in _NC_CACHE:
        _NC_CACHE[key] = _build_nc(scheme, zero_bias=zero_bias)
    return _NC_CACHE[key]


# ------------------------------------------------------------- host helpers
def _host_chain(x, ews, ebs, dws, dbs, in_dtype):
    """Bit-exact replication of the reference EDMD chain on CPU via eager jax.

    The LAPACK eigenvector phase convention (largest component real) is
    discontinuous in the input bits, so the Koopman operator K must match the
    reference bit-for-bit.  Running the identical op sequence eagerly through
    jax-on-CPU achieves that.
    Returns numpy arrays: (y_host, k_efuns, k_evals, k_modes, y_adv, x_adv).
    """
    import jax
    import jax.numpy as jnp

    jax.config.update("jax_enable_x64", True)
    cpu = jax.devices("cpu")[0]

    def mlp(h, ws, bs):
        for i, (w, b) in enumerate(zip(ws, bs)):
            h = h @ w + b
            if i < len(ws) - 1:
                h = jax.nn.relu(h)
        return h

    with jax.default_device(cpu):
        put = lambda a: jax.device_put(np.asarray(a, dtype=in_dtype), cpu)
        xj = put(x)
        ewsj = [put(w) for w in ews]
        ebsj = [put(b) for b in ebs]
        dwsj = [put(w) for w in dws]
        dbsj = [put(b) for b in dbs]

        y = mlp(xj, ewsj, ebsj)                 # [B, T, L]
        yt = jnp.swapaxes(y, 1, 2)              # [B, L, T]
        xt = jnp.swapaxes(xj, 1, 2)             # [B, D, T]

        y_m, y_p = yt[:, :, :-1], yt[:, :, 1:]
        U, S, Vh = jnp.linalg.svd(y_m, full_matrices=False)
        V = jnp.swapaxes(Vh, -1, -2)
        pinv = (V / S[:, None, :]) @ jnp.swapaxes(U, -1, -2)
        Kop = y_p @ pinv
        evals, modes = jnp.linalg.eig(Kop)
        k_evals = jnp.log(evals) / DT_STEP
        k_efuns = jnp.linalg.solve(modes, xt.astype(modes.dtype))
        xint = k_efuns[:, :, -1:]

        def step(e, _):
            yp = (modes @ (e[:, :, None] * xint))[..., 0]
            return e * e, yp

        _, ys = jax.lax.scan(step, k_evals * k_evals, None, length=P_STEPS)
        y_adv = jnp.transpose(ys, (1, 0, 2))    # [B, P, L] complex
        y_adv_real = jnp.real(y_adv)
        x_adv = mlp(y_adv_real, dwsj, dbsj)     # [B, P, D]

    return (np.asarray(y), np.asarray(k_efuns), np.asarray(k_evals),
            np.asarray(modes), np.asarray(y_adv), np.asarray(x_adv))


def _in_maps_for(scheme, x, ws, bs):
    """Build the per-core input maps."""
    maps = []
    w_np = {}
    def _pack(w32):
        # [fin, fout] -> [ksz, nk*fout]: k-chunks side by side in free dim
        fin = w32.shape[0]
        parts = [w32[ks:ks + ksz, :] for (ks, ksz) in _chunks(fin)]
        return np.ascontiguousarray(np.concatenate(parts, axis=1))

    for (lname, fin, fout, _), w in zip(ENC + DEC, ws):
        w32 = np.ascontiguousarray(w, dtype=np.float32)
        if scheme == "r3" and lname == "e0":
            hi, lo = _split_hi_lo(w32)
            w_np["w_e0_p1"] = np.ascontiguousarray(np.concatenate([hi, hi], axis=0))
            w_np["w_e0_p2"] = np.ascontiguousarray(
                np.concatenate([lo, np.zeros_like(lo)], axis=0))
        elif scheme == "r3":
            hi, lo = _split_hi_lo(w32)
            w_np[f"w_{lname}_hi"] = _pack(hi)
            w_np[f"w_{lname}_lo"] = _pack(lo)
        else:
            w_np[f"w_{lname}"] = _pack(w32)
    b_np = {}
    for (lname, fin, fout, _), b in zip(ENC + DEC, bs):
        b_np[f"b_{lname}"] = np.ascontiguousarray(b, dtype=np.float32).reshape(-1, 1)

    for c in range(N_CORES):
        xs = x[c * B_SH:(c + 1) * B_SH].reshape(NTOK, D)
        xT = np.ascontiguousarray(xs.T, dtype=np.float32)
        m = {}
        if scheme == "r3":
            hi, lo = _split_hi_lo(xT)
            m["xT_hilo"] = np.ascontiguousarray(np.concatenate([hi, lo], axis=0))
        else:
            m["xT"] = xT
        m.update(w_np)
        m.update(b_np)
        maps.append(m)
    return maps


def kernel(**inputs):
    x = np.asarray(inputs["x"])
    in_dtype = x.dtype if x.dtype in (np.float32, np.float64) else np.float64
    cdtype = np.complex64 if in_dtype == np.float32 else np.complex128

    ews = [np.asarray(inputs[f"enc_w{i}"]) for i in range(4)]
    ebs = [np.asarray(inputs[f"enc_b{i}"]) for i in range(4)]
    dws = [np.asarray(inputs[f"dec_w{i}"]) for i in range(4)]
    dbs = [np.asarray(inputs[f"dec_b{i}"]) for i in range(4)]

    # ---------------- device: y = enc(x), x_ae = dec(y) ----------------
    scheme = SCHEME
    zero_bias = all(not np.any(b) for b in ebs + dbs)
    nc = _get_nc(scheme, zero_bias)
    in_maps = _in_maps_for(scheme, x.astype(np.float32, copy=False), ews + dws, ebs + dbs)
    # x64 mode makes the bass2jax compile pathological (~15x slower); the
    # device program is pure float32, so pin x64 off around the launch.
    import jax
    x64_prev = bool(jax.config.jax_enable_x64)
    try:
        jax.config.update("jax_enable_x64", False)
        res = run_bass_kernel_spmd(nc, in_maps, core_ids=list(range(N_CORES)))
    finally:
        jax.config.update("jax_enable_x64", x64_prev)

    y_dev = np.empty((B, T, L), dtype=np.float32)
    xae_dev = np.empty((B, T, D), dtype=np.float32)
    for c in range(N_CORES):
        y_dev[c * B_SH:(c + 1) * B_SH] = res.results[c]["yT"].T.reshape(B_SH, T, L)
        xae_dev[c * B_SH:(c + 1) * B_SH] = res.results[c]["xaeT"].T.reshape(B_SH, T, D)

    # ------- host: bit-exact reference EDMD chain in input precision -------
    (_y_host, k_efuns, k_evals, k_modes, y_adv, x_adv) = _host_chain(
        x, ews, ebs, dws, dbs, in_dtype)
    y_adv_real = np.real(y_adv)
    y_adv_imag = np.imag(y_adv)

    return (
        y_dev.astype(in_dtype),
        xae_dev.astype(in_dtype),
        x_adv.astype(in_dtype),
        y_adv_real.astype(in_dtype),
        y_adv_imag.astype(in_dtype),
        k_evals.astype(cdtype),
        k_efuns.astype(cdtype),
        k_modes.astype(cdtype),
    )


# revision 26
# speedup vs baseline: 1.0841x; 1.0761x over previous
"""Trainium2 Bass kernel for the DLEDMD problem.

Contract: kernel(**inputs) takes the FULL unsharded inputs (numpy arrays,
keyed as in setup_inputs()) and returns the FULL output tuple matching
reference():  (y, x_ae, x_adv, y_adv_real, y_adv_imag, k_evals, k_efuns,
k_modes).

Strategy
--------
 * Pure data parallel across 8 NeuronCores: batch 128 -> 16 per core.
 * Device computes the two heavy MLPs (encoder y = mlp(x), decoder
   x_ae = mlp(y)); activations kept TRANSPOSED [features, tokens] on chip
   so no on-device transposes are needed (host transposes in/out).
 * The LAPACK-convention-bound EDMD chain (SVD -> pinv -> eig -> solve ->
   scan) and the tiny [128,4,64] decoder pass run on host in float64,
   reproducing the reference bit-for-bit up to BLAS rounding.
 * Matmul precision scheme (SCHEME):
     - "fp32": native float32 matmuls (4 cycles/row on PE).
     - "r1":   float32r single pass (1 cycle/row, ~11 mantissa bits).
     - "r3":   float32r hi/lo split, 3 passes -> ~fp32 accuracy at 3 cycles.
"""

import os
import sys

import numpy as np

try:
    import concourse.bass as bass  # noqa: F401
except Exception:  # pragma: no cover
    sys.path.insert(0, "/opt/trn_rl_repo")

import concourse.bacc as bacc
import concourse.tile as tile
from concourse import mybir
from concourse.bass_utils import run_bass_kernel_spmd

# ---------------------------------------------------------------- constants
B, T, D, L, H = 128, 256, 64, 64, 256
P_STEPS, DT_STEP = 4, 1.0
N_CORES = 8
B_SH = B // N_CORES            # 16 batches per core
NTOK = B_SH * T                # 4096 tokens per core
NW = 512                       # tokens per PSUM chunk
N_CH = NTOK // NW              # 8 chunks

SCHEME = os.environ.get("DLEDMD_SCHEME", "r3")

# layer table: (name, fin, fout, relu)
ENC = [("e0", D, H, True), ("e1", H, H, True), ("e2", H, H, True), ("e3", H, L, False)]
DEC = [("d0", L, H, True), ("d1", H, H, True), ("d2", H, H, True), ("d3", H, D, False)]

_HI_MASK = np.uint32(0xFFFFF000)  # keep 11 explicit mantissa bits


def _chunks(n):
    """split feature dim n into partition chunks of <=128"""
    out = []
    s = 0
    while s < n:
        c = min(128, n - s)
        out.append((s, c))
        s += c
    return out


def _split_hi_lo(a):
    """split fp32 array into (hi, lo) with 11-bit-mantissa hi via truncation"""
    a = np.ascontiguousarray(a, dtype=np.float32)
    hi = (a.view(np.uint32) & _HI_MASK).view(np.float32)
    lo = ((a - hi).view(np.uint32) & _HI_MASK).view(np.float32)
    return hi, np.ascontiguousarray(lo)


def _build_nc(scheme, repeat=1, zero_bias=True, loop=True,
              x_ring="scalar", store_ring="sync", hilo_bufs=4, ps_bufs=8,
              x_dmas=2, staged_store=False):
    f32 = mybir.dt.float32
    f32r = mybir.dt.float32r
    mm_dt = f32 if scheme == "fp32" else f32r
    act_t = mybir.ActivationFunctionType

    nc = bacc.Bacc("TRN2", target_bir_lowering=False, debug=False)

    # ---------------- DRAM I/O ----------------
    # activations input (transposed) and weights; r3 passes hi/lo pre-split
    def din(name, shape, dt):
        return nc.dram_tensor(name, shape, dt, kind="ExternalInput").ap()

    if scheme == "r3":
        # stacked gen-0 input: rows 0..63 = x_hi, rows 64..127 = x_lo; the
        # e0 weights are K-stacked to match ([w_hi;w_hi] and [w_lo;0]), so
        # layer e0 runs 2 full-K=128 passes instead of 3 half-K=64 ones.
        x_in = [din("xT_hilo", [2 * D, NTOK], mm_dt)]
    else:
        x_in = [din("xT", [D, NTOK], mm_dt)]

    w_in = {}
    b_in = {}
    for lname, fin, fout, _ in ENC + DEC:
        kch = _chunks(fin)
        pshape = [kch[0][1], len(kch) * fout]
        if scheme == "r3" and lname == "e0":
            w_in[lname] = [din("w_e0_p1", [2 * D, fout], mm_dt),
                           din("w_e0_p2", [2 * D, fout], mm_dt)]
        elif scheme == "r3":
            w_in[lname] = [din(f"w_{lname}_hi", pshape, mm_dt),
                           din(f"w_{lname}_lo", pshape, mm_dt)]
        else:
            w_in[lname] = [din(f"w_{lname}", pshape, mm_dt)]
        b_in[lname] = din(f"b_{lname}", [fout, 1], f32)

    y_out = nc.dram_tensor("yT", [L, NTOK], f32, kind="ExternalOutput").ap()
    xae_out = nc.dram_tensor("xaeT", [D, NTOK], f32, kind="ExternalOutput").ap()

    with tile.TileContext(nc) as tc:
        with (
            tc.tile_pool(name="wpool", bufs=1) as wpool,
            tc.tile_pool(name="hi", bufs=hilo_bufs) as hi_pool,
            tc.tile_pool(name="lo", bufs=hilo_bufs) as lo_pool,
            tc.tile_pool(name="full", bufs=6) as full_pool,
            tc.tile_pool(name="outb", bufs=1) as out_pool,
            tc.tile_pool(name="ps", bufs=ps_bufs, space="PSUM") as ps_pool,
        ):
            # ---------- load weights + biases (once, outside any repeat) ----
            # weights arrive host-packed as [ksz, nk*fout] (k-chunks along the
            # free dim) so each (layer, pass) is a single large DMA -- the
            # HWDGE ring's ~0.6us fixed cost per dma_start otherwise starves
            # the PE during the first layers.
            w_sb = {}   # w_sb[lname][pass][ki] -> AP [ksz, fout]
            b_sb = {}   # b_sb[lname][mi] -> tile [msz, 1]
            for lname, fin, fout, _ in ENC + DEC:
                passes = []
                kch = _chunks(fin)
                if scheme == "r3" and lname == "e0":
                    kch = [(0, 2 * D)]
                for wdram in w_in[lname]:
                    wt = wpool.tile([kch[0][1], len(kch) * fout], mm_dt,
                                    tag=f"w_{lname}_{len(passes)}",
                                    name=f"w_{lname}_{len(passes)}")
                    nc.sync.dma_start(out=wt, in_=wdram)
                    passes.append([wt[:ksz, ki * fout:(ki + 1) * fout]
                                   for ki, (ks, ksz) in enumerate(kch)])
                w_sb[lname] = passes
                bts = []
                if not zero_bias:
                    for (ms, msz) in _chunks(fout):
                        bt = wpool.tile([msz, 1], f32, tag=f"b_{lname}_{ms}",
                                        name=f"b_{lname}_{ms}")
                        nc.sync.dma_start(out=bt, in_=b_in[lname][ms:ms + msz, :])
                        bts.append(bt)
                b_sb[lname] = bts

            def body():
                # ---- load x (generation-0 activations) ----
                # one dma_start per tensor on the scalar HWDGE ring: each
                # dma_start costs ~0.6us of serialized ring time, so fewer,
                # larger transfers win; the ring split keeps x off the
                # weight stream.
                eng = nc.scalar if x_ring == "scalar" else nc.sync
                xdim = 2 * D if scheme == "r3" else D
                xt = hi_pool.tile([xdim, NTOK], mm_dt, tag="hi", name="x_t")
                xw = NTOK // x_dmas
                for c in range(x_dmas):
                    csl = slice(c * xw, (c + 1) * xw)
                    eng.dma_start(out=xt[:, csl], in_=x_in[0][:, csl])
                cur_hi = [(xdim, xt)]
                cur_lo = [(xdim, xt)] if scheme == "r3" else None

                store_eng = nc.scalar if store_ring == "scalar" else nc.sync
                y_stage = xae_stage = None
                if staged_store:
                    y_stage = out_pool.tile([L, NTOK], f32, tag="y_stage",
                                            name="y_stage")
                    xae_stage = out_pool.tile([D, NTOK], f32, tag="xae_stage",
                                              name="xae_stage")

                evac_flip = 0  # alternate ACT / DVE for relu evacuation
                for li, (lname, fin, fout, relu) in enumerate(ENC + DEC):
                    is_enc_out = lname == "e3"   # y, needed as decoder input
                    is_final = not relu
                    mchunks = _chunks(fout)
                    kchunks = _chunks(fin)
                    # matmul passes: r3 (w_hi,a_hi),(w_hi,a_lo),(w_lo,a_hi);
                    # e0 is K-stacked: both passes consume the same rhs
                    if scheme == "r3" and lname == "e0":
                        pair_sel = [(0, cur_hi), (1, cur_hi)]
                        kchunks = [(0, 2 * D)]
                    elif scheme == "r3":
                        pair_sel = [(0, cur_hi), (0, cur_lo), (1, cur_hi)]
                    else:
                        pair_sel = [(0, cur_hi)]

                    new_hi = []
                    new_lo = []
                    if (not is_final) or is_enc_out:
                        for (ms, msz) in mchunks:
                            new_hi.append((msz, hi_pool.tile(
                                [msz, NTOK], mm_dt, tag="hi", name=f"h_{lname}_{ms}")))
                            if scheme == "r3":
                                new_lo.append((msz, lo_pool.tile(
                                    [msz, NTOK], mm_dt, tag="lo", name=f"hlo_{lname}_{ms}")))

                    for mi, (ms, msz) in enumerate(mchunks):
                        bias = None if zero_bias else b_sb[lname][mi]
                        for n in range(N_CH):
                            nsl = slice(n * NW, (n + 1) * NW)
                            ps = ps_pool.tile([msz, NW], f32, tag="ps",
                                              name=f"ps_{lname}_{ms}_{n}")
                            mms = []
                            for (wp, acts) in pair_sel:
                                for ki, (ks, ksz) in enumerate(kchunks):
                                    mms.append((w_sb[lname][wp][ki][:, ms:ms + msz],
                                                acts[ki][1][:, nsl]))
                            for i, (lhsT, rhs) in enumerate(mms):
                                nc.tensor.matmul(ps, lhsT, rhs,
                                                 start=(i == 0), stop=(i == len(mms) - 1))
                            # ----- evacuate PSUM -----
                            if not is_final:
                                if scheme == "r3" and zero_bias:
                                    # 2-op evac: hi = RN(relu(ps)) on ACT,
                                    # lo = relu(ps) - hi fused on DVE
                                    nc.scalar.activation(
                                        new_hi[mi][1][:, nsl], ps, act_t.Relu)
                                    nc.vector.scalar_tensor_tensor(
                                        out=new_lo[mi][1][:, nsl], in0=ps,
                                        scalar=0.0,
                                        in1=new_hi[mi][1][:, nsl].bitcast(f32),
                                        op0=mybir.AluOpType.max,
                                        op1=mybir.AluOpType.subtract)
                                elif scheme == "r3":
                                    fullt = full_pool.tile(
                                        [msz, NW], f32, tag="full",
                                        name=f"full_{lname}_{ms}_{n}")
                                    nc.scalar.activation(fullt, ps, act_t.Relu, bias=bias)
                                    # hi = round_to_f32r(full) (DVE write-rounds)
                                    nc.vector.tensor_copy(new_hi[mi][1][:, nsl], fullt)
                                    # lo = full - hi (f32r write-round harmless)
                                    nc.vector.tensor_tensor(
                                        out=new_lo[mi][1][:, nsl], in0=fullt,
                                        in1=new_hi[mi][1][:, nsl].bitcast(f32),
                                        op=mybir.AluOpType.subtract)
                                else:
                                    dst = new_hi[mi][1][:, nsl]
                                    bias_arg = 0.0 if bias is None else bias
                                    if evac_flip % 2 == 0:
                                        nc.scalar.activation(dst, ps, act_t.Relu,
                                                             bias=bias_arg)
                                    else:
                                        nc.vector.tensor_scalar(
                                            out=dst, in0=ps, scalar1=bias_arg,
                                            scalar2=0.0,
                                            op0=mybir.AluOpType.add,
                                            op1=mybir.AluOpType.max)
                                    evac_flip += 1
                            else:
                                if staged_store:
                                    stage = y_stage if is_enc_out else xae_stage
                                    outt = stage[ms:ms + msz, nsl]
                                else:
                                    outt = out_pool.tile(
                                        [msz, NW], f32, tag="outb",
                                        name=f"out_{lname}_{ms}_{n}", bufs=6)
                                if zero_bias:
                                    nc.scalar.activation(outt, ps, act_t.Copy)
                                else:
                                    nc.scalar.activation(outt, ps, act_t.Identity,
                                                         bias=bias)
                                if not staged_store:
                                    dram = y_out if is_enc_out else xae_out
                                    store_eng.dma_start(
                                        out=dram[ms:ms + msz, nsl], in_=outt)
                                if is_enc_out:
                                    # decoder consumes y as f32r (+ lo for r3)
                                    src_ap = ps if zero_bias else outt
                                    nc.vector.tensor_copy(new_hi[mi][1][:, nsl], src_ap)
                                    if scheme == "r3":
                                        nc.vector.scalar_tensor_tensor(
                                            out=new_lo[mi][1][:, nsl], in0=src_ap,
                                            scalar=0.0,
                                            in1=new_hi[mi][1][:, nsl].bitcast(f32),
                                            op0=mybir.AluOpType.add,
                                            op1=mybir.AluOpType.subtract)

                    if (not is_final) or is_enc_out:
                        cur_hi = new_hi
                        cur_lo = new_lo if scheme == "r3" else None
                    if staged_store and lname == "e3":
                        store_eng.dma_start(out=y_out, in_=y_stage)
                    elif staged_store and lname == "d3":
                        store_eng.dma_start(out=xae_out, in_=xae_stage)

            # Wrap in a hardware loop (even for repeat=1): walrus's embedded
            # birsim skips dynamic loops, cutting NEFF compile time from
            # ~6 min to ~3 s for the flat body at ~us runtime cost.
            if loop:
                import concourse.mybir as _mb
                kw = {}
                if repeat > 1:
                    # cheaper back-edge for benchmarking loops
                    kw = dict(hint_engines=(_mb.EngineType.PE,
                                            _mb.EngineType.Activation,
                                            _mb.EngineType.DVE))
                with tc.For_i(0, repeat, 1, **kw):
                    body()
            else:
                body()

    nc.finalize()
    return nc


_NC_CACHE = {}


def _get_nc(scheme, zero_bias=True):
    key = (scheme, zero_bias)
    if key not in _NC_CACHE:
        _NC_CACHE[key] = _build_nc(scheme, zero_bias=zero_bias)
    return _NC_CACHE[key]


# ------------------------------------------------------------- host helpers
def _host_chain(x, ews, ebs, dws, dbs, in_dtype):
    """Bit-exact replication of the reference EDMD chain on CPU via eager jax.

    The LAPACK eigenvector phase convention (largest component real) is
    discontinuous in the input bits, so the Koopman operator K must match the
    reference bit-for-bit.  Running the identical op sequence eagerly through
    jax-on-CPU achieves that.
    Returns numpy arrays: (y_host, k_efuns, k_evals, k_modes, y_adv, x_adv).
    """
    import jax
    import jax.numpy as jnp

    jax.config.update("jax_enable_x64", True)
    cpu = jax.devices("cpu")[0]

    def mlp(h, ws, bs):
        for i, (w, b) in enumerate(zip(ws, bs)):
            h = h @ w + b
            if i < len(ws) - 1:
                h = jax.nn.relu(h)
        return h

    with jax.default_device(cpu):
        put = lambda a: jax.device_put(np.asarray(a, dtype=in_dtype), cpu)
        xj = put(x)
        ewsj = [put(w) for w in ews]
        ebsj = [put(b) for b in ebs]
        dwsj = [put(w) for w in dws]
        dbsj = [put(b) for b in dbs]

        y = mlp(xj, ewsj, ebsj)                 # [B, T, L]
        yt = jnp.swapaxes(y, 1, 2)              # [B, L, T]
        xt = jnp.swapaxes(xj, 1, 2)             # [B, D, T]

        y_m, y_p = yt[:, :, :-1], yt[:, :, 1:]
        U, S, Vh = jnp.linalg.svd(y_m, full_matrices=False)
        V = jnp.swapaxes(Vh, -1, -2)
        pinv = (V / S[:, None, :]) @ jnp.swapaxes(U, -1, -2)
        Kop = y_p @ pinv
        evals, modes = jnp.linalg.eig(Kop)
        k_evals = jnp.log(evals) / DT_STEP
        k_efuns = jnp.linalg.solve(modes, xt.astype(modes.dtype))
        xint = k_efuns[:, :, -1:]

        def step(e, _):
            yp = (modes @ (e[:, :, None] * xint))[..., 0]
            return e * e, yp

        _, ys = jax.lax.scan(step, k_evals * k_evals, None, length=P_STEPS)
        y_adv = jnp.transpose(ys, (1, 0, 2))    # [B, P, L] complex
        y_adv_real = jnp.real(y_adv)
        x_adv = mlp(y_adv_real, dwsj, dbsj)     # [B, P, D]

    return (np.asarray(y), np.asarray(k_efuns), np.asarray(k_evals),
            np.asarray(modes), np.asarray(y_adv), np.asarray(x_adv))


def _in_maps_for(scheme, x, ws, bs):
    """Build the per-core input maps."""
    maps = []
    w_np = {}
    def _pack(w32):
        # [fin, fout] -> [ksz, nk*fout]: k-chunks side by side in free dim
        fin = w32.shape[0]
        parts = [w32[ks:ks + ksz, :] for (ks, ksz) in _chunks(fin)]
        return np.ascontiguousarray(np.concatenate(parts, axis=1))

    for (lname, fin, fout, _), w in zip(ENC + DEC, ws):
        w32 = np.ascontiguousarray(w, dtype=np.float32)
        if scheme == "r3" and lname == "e0":
            hi, lo = _split_hi_lo(w32)
            w_np["w_e0_p1"] = np.ascontiguousarray(np.concatenate([hi, hi], axis=0))
            w_np["w_e0_p2"] = np.ascontiguousarray(
                np.concatenate([lo, np.zeros_like(lo)], axis=0))
        elif scheme == "r3":
            hi, lo = _split_hi_lo(w32)
            w_np[f"w_{lname}_hi"] = _pack(hi)
            w_np[f"w_{lname}_lo"] = _pack(lo)
        else:
            w_np[f"w_{lname}"] = _pack(w32)
    b_np = {}
    for (lname, fin, fout, _), b in zip(ENC + DEC, bs):
        b_np[f"b_{lname}"] = np.ascontiguousarray(b, dtype=np.float32).reshape(-1, 1)

    for c in range(N_CORES):
        xs = x[c * B_SH:(c + 1) * B_SH].reshape(NTOK, D)
        xT = np.ascontiguousarray(xs.T, dtype=np.float32)
        m = {}
        if scheme == "r3":
            hi, lo = _split_hi_lo(xT)
            m["xT_hilo"] = np.ascontiguousarray(np.concatenate([hi, lo], axis=0))
        else:
            m["xT"] = xT
        m.update(w_np)
        m.update(b_np)
        maps.append(m)
    return maps


def kernel(**inputs):
    x = np.asarray(inputs["x"])
    in_dtype = x.dtype if x.dtype in (np.float32, np.float64) else np.float64
    cdtype = np.complex64 if in_dtype == np.float32 else np.complex128

    ews = [np.asarray(inputs[f"enc_w{i}"]) for i in range(4)]
    ebs = [np.asarray(inputs[f"enc_b{i}"]) for i in range(4)]
    dws = [np.asarray(inputs[f"dec_w{i}"]) for i in range(4)]
    dbs = [np.asarray(inputs[f"dec_b{i}"]) for i in range(4)]

    # ---------------- device: y = enc(x), x_ae = dec(y) ----------------
    scheme = SCHEME
    zero_bias = all(not np.any(b) for b in ebs + dbs)
    nc = _get_nc(scheme, zero_bias)
    in_maps = _in_maps_for(scheme, x.astype(np.float32, copy=False), ews + dws, ebs + dbs)
    # x64 mode makes the bass2jax compile pathological (~15x slower); the
    # device program is pure float32, so pin x64 off around the launch.
    import jax
    x64_prev = bool(jax.config.jax_enable_x64)
    try:
        jax.config.update("jax_enable_x64", False)
        res = run_bass_kernel_spmd(nc, in_maps, core_ids=list(range(N_CORES)))
    finally:
        jax.config.update("jax_enable_x64", x64_prev)

    y_dev = np.empty((B, T, L), dtype=np.float32)
    xae_dev = np.empty((B, T, D), dtype=np.float32)
    for c in range(N_CORES):
        y_dev[c * B_SH:(c + 1) * B_SH] = res.results[c]["yT"].T.reshape(B_SH, T, L)
        xae_dev[c * B_SH:(c + 1) * B_SH] = res.results[c]["xaeT"].T.reshape(B_SH, T, D)

    # ------- host: bit-exact reference EDMD chain in input precision -------
    (_y_host, k_efuns, k_evals, k_modes, y_adv, x_adv) = _host_chain(
        x, ews, ebs, dws, dbs, in_dtype)
    y_adv_real = np.real(y_adv)
    y_adv_imag = np.imag(y_adv)

    return (
        y_dev.astype(in_dtype),
        xae_dev.astype(in_dtype),
        x_adv.astype(in_dtype),
        y_adv_real.astype(in_dtype),
        y_adv_imag.astype(in_dtype),
        k_evals.astype(cdtype),
        k_efuns.astype(cdtype),
        k_modes.astype(cdtype),
    )


# revision 36
# speedup vs baseline: 1.2891x; 1.1891x over previous
"""Trainium2 Bass kernel for the DLEDMD problem.

Contract: kernel(**inputs) takes the FULL unsharded inputs (numpy arrays,
keyed as in setup_inputs()) and returns the FULL output tuple matching
reference():  (y, x_ae, x_adv, y_adv_real, y_adv_imag, k_evals, k_efuns,
k_modes).

Strategy
--------
 * Pure data parallel across 8 NeuronCores: batch 128 -> 16 per core.
 * Device computes the two heavy MLPs (encoder y = mlp(x), decoder
   x_ae = mlp(y)); activations kept TRANSPOSED [features, tokens] on chip
   so no on-device transposes are needed (host transposes in/out).
 * The LAPACK-convention-bound EDMD chain (SVD -> pinv -> eig -> solve ->
   scan) and the tiny [128,4,64] decoder pass run on host in float64,
   reproducing the reference bit-for-bit up to BLAS rounding.
 * Matmul precision scheme (SCHEME):
     - "fp32": native float32 matmuls (4 cycles/row on PE).
     - "r1":   float32r single pass (1 cycle/row, ~11 mantissa bits).
     - "r3":   float32r hi/lo split, 3 passes -> ~fp32 accuracy at 3 cycles.
"""

import os
import sys

import numpy as np

try:
    import concourse.bass as bass  # noqa: F401
except Exception:  # pragma: no cover
    sys.path.insert(0, "/opt/trn_rl_repo")

import concourse.bacc as bacc
import concourse.tile as tile
from concourse import mybir
from concourse.bass_utils import run_bass_kernel_spmd

# ---------------------------------------------------------------- constants
B, T, D, L, H = 128, 256, 64, 64, 256
P_STEPS, DT_STEP = 4, 1.0
N_CORES = 8
B_SH = B // N_CORES            # 16 batches per core
NTOK = B_SH * T                # 4096 tokens per core
NW = 512                       # tokens per PSUM chunk
N_CH = NTOK // NW              # 8 chunks

SCHEME = os.environ.get("DLEDMD_SCHEME", "r3")

# layer table: (name, fin, fout, relu)
ENC = [("e0", D, H, True), ("e1", H, H, True), ("e2", H, H, True), ("e3", H, L, False)]
DEC = [("d0", L, H, True), ("d1", H, H, True), ("d2", H, H, True), ("d3", H, D, False)]

_HI_MASK = np.uint32(0xFFFFF000)  # keep 11 explicit mantissa bits


def _chunks(n):
    """split feature dim n into partition chunks of <=128"""
    out = []
    s = 0
    while s < n:
        c = min(128, n - s)
        out.append((s, c))
        s += c
    return out


def _split_hi_lo(a):
    """split fp32 array into (hi, lo) with 11-bit-mantissa hi via truncation"""
    a = np.ascontiguousarray(a, dtype=np.float32)
    hi = (a.view(np.uint32) & _HI_MASK).view(np.float32)
    lo = ((a - hi).view(np.uint32) & _HI_MASK).view(np.float32)
    return hi, np.ascontiguousarray(lo)


def _build_nc(scheme, repeat=1, zero_bias=True, loop=True,
              x_ring="scalar", store_ring="sync", hilo_bufs=4, ps_bufs=8,
              x_dmas=2, staged_store=False, warmup_mms=8, x_head=0, ev_w=512):
    f32 = mybir.dt.float32
    f32r = mybir.dt.float32r
    mm_dt = f32 if scheme == "fp32" else f32r
    act_t = mybir.ActivationFunctionType

    nc = bacc.Bacc("TRN2", target_bir_lowering=False, debug=False)

    # ---------------- DRAM I/O ----------------
    # activations input (transposed) and weights; r3 passes hi/lo pre-split
    def din(name, shape, dt):
        return nc.dram_tensor(name, shape, dt, kind="ExternalInput").ap()

    if scheme == "r3":
        # stacked gen-0 input: rows 0..63 = x_hi, rows 64..127 = x_lo; the
        # e0 weights are K-stacked to match ([w_hi;w_hi] and [w_lo;0]), so
        # layer e0 runs 2 full-K=128 passes instead of 3 half-K=64 ones.
        x_in = [din("xT_hilo", [2 * D, NTOK], mm_dt)]
    else:
        x_in = [din("xT", [D, NTOK], mm_dt)]

    w_in = {}
    b_in = {}
    for lname, fin, fout, _ in ENC + DEC:
        kch = _chunks(fin)
        pshape = [kch[0][1], len(kch) * fout]
        if scheme == "r3" and lname in ("e0", "d0") and zero_bias:
            # K-stacked passes consume the stacked [hi; lo] activation tile
            w_in[lname] = [din(f"w_{lname}_p1", [2 * D, fout], mm_dt),
                           din(f"w_{lname}_p2", [2 * D, fout], mm_dt)]
        elif scheme == "r3" and lname == "e0":
            w_in[lname] = [din("w_e0_p1", [2 * D, fout], mm_dt),
                           din("w_e0_p2", [2 * D, fout], mm_dt)]
        elif scheme == "r3" and lname == "e3" and zero_bias:
            # output columns duplicated: psum rows 0-63 and 64-127 both = y
            w_in[lname] = [din(f"w_{lname}_hi", [kch[0][1], len(kch) * 2 * L], mm_dt),
                           din(f"w_{lname}_lo", [kch[0][1], len(kch) * 2 * L], mm_dt)]
        elif scheme == "r3":
            w_in[lname] = [din(f"w_{lname}_hi", pshape, mm_dt),
                           din(f"w_{lname}_lo", pshape, mm_dt)]
        else:
            w_in[lname] = [din(f"w_{lname}", pshape, mm_dt)]
        b_in[lname] = din(f"b_{lname}", [fout, 1], f32)

    y_out = nc.dram_tensor("yT", [L, NTOK], f32, kind="ExternalOutput").ap()
    xae_out = nc.dram_tensor("xaeT", [D, NTOK], f32, kind="ExternalOutput").ap()

    with tile.TileContext(nc) as tc:
        with (
            tc.tile_pool(name="wpool", bufs=1) as wpool,
            tc.tile_pool(name="hi", bufs=hilo_bufs) as hi_pool,
            tc.tile_pool(name="lo", bufs=hilo_bufs) as lo_pool,
            tc.tile_pool(name="full", bufs=6) as full_pool,
            tc.tile_pool(name="outb", bufs=1) as out_pool,
            tc.tile_pool(name="ps", bufs=ps_bufs, space="PSUM") as ps_pool,
        ):
            # ---------- load weights + biases (once, outside any repeat) ----
            # weights arrive host-packed as [ksz, nk*fout] (k-chunks along the
            # free dim) so each (layer, pass) is a single large DMA -- the
            # HWDGE ring's ~0.6us fixed cost per dma_start otherwise starves
            # the PE during the first layers.
            w_sb = {}   # w_sb[lname][pass][ki] -> AP [ksz, fout]
            b_sb = {}   # b_sb[lname][mi] -> tile [msz, 1]
            for lname, fin, fout, _ in ENC + DEC:
                passes = []
                kch = _chunks(fin)
                fout_mm = fout
                if scheme == "r3" and lname == "e0":
                    kch = [(0, 2 * D)]
                if scheme == "r3" and lname == "d0" and zero_bias:
                    kch = [(0, 2 * D)]
                if scheme == "r3" and lname == "e3" and zero_bias:
                    fout_mm = 2 * L
                for wdram in w_in[lname]:
                    wt = wpool.tile([kch[0][1], len(kch) * fout_mm], mm_dt,
                                    tag=f"w_{lname}_{len(passes)}",
                                    name=f"w_{lname}_{len(passes)}")
                    nc.sync.dma_start(out=wt, in_=wdram)
                    passes.append([wt[:ksz, ki * fout_mm:(ki + 1) * fout_mm]
                                   for ki, (ks, ksz) in enumerate(kch)])
                w_sb[lname] = passes
                bts = []
                if not zero_bias:
                    for (ms, msz) in _chunks(fout):
                        bt = wpool.tile([msz, 1], f32, tag=f"b_{lname}_{ms}",
                                        name=f"b_{lname}_{ms}")
                        nc.sync.dma_start(out=bt, in_=b_in[lname][ms:ms + msz, :])
                        bts.append(bt)
                b_sb[lname] = bts

            def body():
                # ---- load x (generation-0 activations) ----
                # one dma_start per tensor on the scalar HWDGE ring: each
                # dma_start costs ~0.6us of serialized ring time, so fewer,
                # larger transfers win; the ring split keeps x off the
                # weight stream.
                eng = nc.scalar if x_ring == "scalar" else nc.sync
                xdim = 2 * D if scheme == "r3" else D

                if warmup_mms:
                    # PE warm-up: the HAM clock gate holds the PE at 1.2 GHz
                    # until ~3.4us of sustained activity; burn the DMA-head
                    # idle time on junk matmuls (reusing the first weight tile,
                    # result never read) so the real layers run warm.
                    wz = w_sb["e0"][0][0]
                    wps = ps_pool.tile([128, NW // 2], f32, tag="ps",
                                       name="warm_ps")
                    for wi in range(warmup_mms):
                        nc.tensor.matmul(wps, wz[:, :128], wz[:, :NW // 2],
                                         start=(wi == 0),
                                         stop=(wi == warmup_mms - 1))

                xt = hi_pool.tile([xdim, NTOK], mm_dt, tag="hi", name="x_t")
                bounds = [0]
                if x_head:
                    bounds.append(x_head)
                rem = NTOK - bounds[-1]
                for c in range(x_dmas):
                    bounds.append(bounds[-1] + rem // x_dmas)
                bounds[-1] = NTOK
                for c in range(len(bounds) - 1):
                    csl = slice(bounds[c], bounds[c + 1])
                    eng.dma_start(out=xt[:, csl], in_=x_in[0][:, csl])
                cur_hi = [(xdim, xt)]
                cur_lo = [(xdim, xt)] if scheme == "r3" else None

                store_eng = nc.scalar if store_ring == "scalar" else nc.sync
                y_stage = xae_stage = None
                if staged_store:
                    y_stage = out_pool.tile([L, NTOK], f32, tag="y_stage",
                                            name="y_stage")
                    xae_stage = out_pool.tile([D, NTOK], f32, tag="xae_stage",
                                              name="xae_stage")

                evac_flip = 0  # alternate ACT / DVE for relu evacuation
                for li, (lname, fin, fout, relu) in enumerate(ENC + DEC):
                    is_enc_out = lname == "e3"   # y, needed as decoder input
                    is_final = not relu
                    mchunks = _chunks(fout)
                    kchunks = _chunks(fin)
                    # matmul passes: r3 (w_hi,a_hi),(w_hi,a_lo),(w_lo,a_hi);
                    # e0 (and d0 under zero_bias) are K-stacked: 2 passes on
                    # a [hi; lo] stacked rhs.  e3 under zero_bias duplicates
                    # its output columns so y appears on both lane halves.
                    stack_y = scheme == "r3" and zero_bias
                    if scheme == "r3" and (lname == "e0" or
                                           (lname == "d0" and stack_y)):
                        pair_sel = [(0, cur_hi), (1, cur_hi)]
                        kchunks = [(0, 2 * D)]
                    elif scheme == "r3":
                        pair_sel = [(0, cur_hi), (0, cur_lo), (1, cur_hi)]
                    else:
                        pair_sel = [(0, cur_hi)]
                    if lname == "e3" and stack_y:
                        mchunks = [(0, 2 * L)]

                    new_hi = []
                    new_lo = []
                    if is_enc_out and stack_y:
                        yst = hi_pool.tile([2 * L, NTOK], mm_dt, tag="hi",
                                           name="y_stacked")
                        new_hi.append((2 * L, yst))
                        new_lo.append((2 * L, yst))
                    elif (not is_final) or is_enc_out:
                        for (ms, msz) in mchunks:
                            new_hi.append((msz, hi_pool.tile(
                                [msz, NTOK], mm_dt, tag="hi", name=f"h_{lname}_{ms}")))
                            if scheme == "r3":
                                new_lo.append((msz, lo_pool.tile(
                                    [msz, NTOK], mm_dt, tag="lo", name=f"hlo_{lname}_{ms}")))

                    n_per_ev = max(1, ev_w // NW)
                    for mi, (ms, msz) in enumerate(mchunks):
                        bias = None if zero_bias else b_sb[lname][mi]
                        for ne in range(N_CH // n_per_ev):
                            # one psum tile spans n_per_ev matmul groups; a
                            # single evacuation then covers ev_w columns,
                            # halving the per-op fixed costs on ACT/DVE
                            evl = ne * n_per_ev * NW
                            nsl = slice(evl, evl + n_per_ev * NW)
                            ps = ps_pool.tile([msz, n_per_ev * NW], f32, tag="ps",
                                              name=f"ps_{lname}_{ms}_{ne}")
                            for half in range(n_per_ev):
                                hsl = slice(evl + half * NW, evl + (half + 1) * NW)
                                psh = ps[:, half * NW:(half + 1) * NW]
                                mms = []
                                for (wp, acts) in pair_sel:
                                    for ki, (ks, ksz) in enumerate(kchunks):
                                        mms.append(
                                            (w_sb[lname][wp][ki][:, ms:ms + msz],
                                             acts[ki][1][:, hsl]))
                                for i, (lhsT, rhs) in enumerate(mms):
                                    nc.tensor.matmul(psh, lhsT, rhs,
                                                     start=(i == 0),
                                                     stop=(i == len(mms) - 1))
                            # ----- evacuate PSUM -----
                            if not is_final:
                                if scheme == "r3" and zero_bias:
                                    # 2-op evac: hi = RN(relu(ps)) on ACT,
                                    # lo = relu(ps) - hi fused on DVE
                                    nc.scalar.activation(
                                        new_hi[mi][1][:, nsl], ps, act_t.Relu)
                                    nc.vector.scalar_tensor_tensor(
                                        out=new_lo[mi][1][:, nsl], in0=ps,
                                        scalar=0.0,
                                        in1=new_hi[mi][1][:, nsl].bitcast(f32),
                                        op0=mybir.AluOpType.max,
                                        op1=mybir.AluOpType.subtract)
                                elif scheme == "r3":
                                    fullt = full_pool.tile(
                                        [msz, n_per_ev * NW], f32, tag="full",
                                        name=f"full_{lname}_{ms}_{ne}")
                                    nc.scalar.activation(fullt, ps, act_t.Relu, bias=bias)
                                    # hi = round_to_f32r(full) (DVE write-rounds)
                                    nc.vector.tensor_copy(new_hi[mi][1][:, nsl], fullt)
                                    # lo = full - hi (f32r write-round harmless)
                                    nc.vector.tensor_tensor(
                                        out=new_lo[mi][1][:, nsl], in0=fullt,
                                        in1=new_hi[mi][1][:, nsl].bitcast(f32),
                                        op=mybir.AluOpType.subtract)
                                else:
                                    dst = new_hi[mi][1][:, nsl]
                                    bias_arg = 0.0 if bias is None else bias
                                    if evac_flip % 2 == 0:
                                        nc.scalar.activation(dst, ps, act_t.Relu,
                                                             bias=bias_arg)
                                    else:
                                        nc.vector.tensor_scalar(
                                            out=dst, in0=ps, scalar1=bias_arg,
                                            scalar2=0.0,
                                            op0=mybir.AluOpType.add,
                                            op1=mybir.AluOpType.max)
                                    evac_flip += 1
                            elif is_enc_out and stack_y:
                                # psum rows 0-63 and 64-127 both hold y.
                                yst = new_hi[0][1]
                                outt = out_pool.tile([L, n_per_ev * NW], f32,
                                                     tag="outb",
                                                     name=f"out_{lname}_{ne}",
                                                     bufs=6)
                                nc.scalar.activation(outt, ps[:L], act_t.Copy)
                                store_eng.dma_start(out=y_out[:, nsl], in_=outt)
                                # hi -> lanes 0-63 (DVE write rounds to f32r)
                                nc.vector.tensor_copy(yst[:L, nsl], ps[:L])
                                # lanes 64-127: round there too, then in-place
                                # lo = y - round(y), all lane-aligned
                                nc.scalar.activation(yst[L:, nsl], ps[L:],
                                                     act_t.Copy)
                                nc.vector.tensor_tensor(
                                    out=yst[L:, nsl], in0=ps[L:],
                                    in1=yst[L:, nsl].bitcast(f32),
                                    op=mybir.AluOpType.subtract)
                            else:
                                if staged_store:
                                    stage = y_stage if is_enc_out else xae_stage
                                    outt = stage[ms:ms + msz, nsl]
                                else:
                                    outt = out_pool.tile(
                                        [msz, n_per_ev * NW], f32, tag="outb",
                                        name=f"out_{lname}_{ms}_{ne}", bufs=6)
                                if zero_bias:
                                    nc.scalar.activation(outt, ps, act_t.Copy)
                                else:
                                    nc.scalar.activation(outt, ps, act_t.Identity,
                                                         bias=bias)
                                if not staged_store:
                                    dram = y_out if is_enc_out else xae_out
                                    store_eng.dma_start(
                                        out=dram[ms:ms + msz, nsl], in_=outt)
                                if is_enc_out:
                                    # decoder consumes y as f32r (+ lo for r3)
                                    src_ap = ps if zero_bias else outt
                                    nc.vector.tensor_copy(new_hi[mi][1][:, nsl], src_ap)
                                    if scheme == "r3":
                                        nc.vector.scalar_tensor_tensor(
                                            out=new_lo[mi][1][:, nsl], in0=src_ap,
                                            scalar=0.0,
                                            in1=new_hi[mi][1][:, nsl].bitcast(f32),
                                            op0=mybir.AluOpType.add,
                                            op1=mybir.AluOpType.subtract)

                    if (not is_final) or is_enc_out:
                        cur_hi = new_hi
                        cur_lo = new_lo if scheme == "r3" else None
                    if staged_store and lname == "e3":
                        store_eng.dma_start(out=y_out, in_=y_stage)
                    elif staged_store and lname == "d3":
                        store_eng.dma_start(out=xae_out, in_=xae_stage)

            # Wrap in a hardware loop (even for repeat=1): walrus's embedded
            # birsim skips dynamic loops, cutting NEFF compile time from
            # ~6 min to ~3 s for the flat body at ~us runtime cost.
            if loop:
                import concourse.mybir as _mb
                kw = {}
                if repeat > 1:
                    # cheaper back-edge for benchmarking loops
                    kw = dict(hint_engines=(_mb.EngineType.PE,
                                            _mb.EngineType.Activation,
                                            _mb.EngineType.DVE))
                with tc.For_i(0, repeat, 1, **kw):
                    body()
            else:
                body()

    nc.finalize()
    return nc


_NC_CACHE = {}


def _get_nc(scheme, zero_bias=True):
    key = (scheme, zero_bias)
    if key not in _NC_CACHE:
        _NC_CACHE[key] = _build_nc(scheme, zero_bias=zero_bias)
    return _NC_CACHE[key]


# ------------------------------------------------------------- host helpers
def _host_chain(x, ews, ebs, dws, dbs, in_dtype):
    """Bit-exact replication of the reference EDMD chain on CPU via eager jax.

    The LAPACK eigenvector phase convention (largest component real) is
    discontinuous in the input bits, so the Koopman operator K must match the
    reference bit-for-bit.  Running the identical op sequence eagerly through
    jax-on-CPU achieves that.
    Returns numpy arrays: (y_host, k_efuns, k_evals, k_modes, y_adv, x_adv).
    """
    import jax
    import jax.numpy as jnp

    jax.config.update("jax_enable_x64", True)
    cpu = jax.devices("cpu")[0]

    def mlp(h, ws, bs):
        for i, (w, b) in enumerate(zip(ws, bs)):
            h = h @ w + b
            if i < len(ws) - 1:
                h = jax.nn.relu(h)
        return h

    with jax.default_device(cpu):
        put = lambda a: jax.device_put(np.asarray(a, dtype=in_dtype), cpu)
        xj = put(x)
        ewsj = [put(w) for w in ews]
        ebsj = [put(b) for b in ebs]
        dwsj = [put(w) for w in dws]
        dbsj = [put(b) for b in dbs]

        y = mlp(xj, ewsj, ebsj)                 # [B, T, L]
        yt = jnp.swapaxes(y, 1, 2)              # [B, L, T]
        xt = jnp.swapaxes(xj, 1, 2)             # [B, D, T]

        y_m, y_p = yt[:, :, :-1], yt[:, :, 1:]
        U, S, Vh = jnp.linalg.svd(y_m, full_matrices=False)
        V = jnp.swapaxes(Vh, -1, -2)
        pinv = (V / S[:, None, :]) @ jnp.swapaxes(U, -1, -2)
        Kop = y_p @ pinv
        evals, modes = jnp.linalg.eig(Kop)
        k_evals = jnp.log(evals) / DT_STEP
        k_efuns = jnp.linalg.solve(modes, xt.astype(modes.dtype))
        xint = k_efuns[:, :, -1:]

        def step(e, _):
            yp = (modes @ (e[:, :, None] * xint))[..., 0]
            return e * e, yp

        _, ys = jax.lax.scan(step, k_evals * k_evals, None, length=P_STEPS)
        y_adv = jnp.transpose(ys, (1, 0, 2))    # [B, P, L] complex
        y_adv_real = jnp.real(y_adv)
        x_adv = mlp(y_adv_real, dwsj, dbsj)     # [B, P, D]

    return (np.asarray(y), np.asarray(k_efuns), np.asarray(k_evals),
            np.asarray(modes), np.asarray(y_adv), np.asarray(x_adv))


def _in_maps_for(scheme, x, ws, bs, zero_bias=True):
    """Build the per-core input maps."""
    maps = []
    w_np = {}
    def _pack(w32):
        # [fin, fout] -> [ksz, nk*fout]: k-chunks side by side in free dim
        fin = w32.shape[0]
        parts = [w32[ks:ks + ksz, :] for (ks, ksz) in _chunks(fin)]
        return np.ascontiguousarray(np.concatenate(parts, axis=1))

    for (lname, fin, fout, _), w in zip(ENC + DEC, ws):
        w32 = np.ascontiguousarray(w, dtype=np.float32)
        if scheme == "r3" and (lname == "e0" or (lname == "d0" and zero_bias)):
            hi, lo = _split_hi_lo(w32)
            w_np[f"w_{lname}_p1"] = np.ascontiguousarray(
                np.concatenate([hi, hi], axis=0))
            w_np[f"w_{lname}_p2"] = np.ascontiguousarray(
                np.concatenate([lo, np.zeros_like(lo)], axis=0))
        elif scheme == "r3" and lname == "e3" and zero_bias:
            aug = np.concatenate([w32, w32], axis=1)   # [256, 128]
            hi, lo = _split_hi_lo(aug)
            w_np[f"w_{lname}_hi"] = _pack(hi)
            w_np[f"w_{lname}_lo"] = _pack(lo)
        elif scheme == "r3":
            hi, lo = _split_hi_lo(w32)
            w_np[f"w_{lname}_hi"] = _pack(hi)
            w_np[f"w_{lname}_lo"] = _pack(lo)
        else:
            w_np[f"w_{lname}"] = _pack(w32)
    b_np = {}
    for (lname, fin, fout, _), b in zip(ENC + DEC, bs):
        b_np[f"b_{lname}"] = np.ascontiguousarray(b, dtype=np.float32).reshape(-1, 1)

    for c in range(N_CORES):
        xs = x[c * B_SH:(c + 1) * B_SH].reshape(NTOK, D)
        xT = np.ascontiguousarray(xs.T, dtype=np.float32)
        m = {}
        if scheme == "r3":
            hi, lo = _split_hi_lo(xT)
            m["xT_hilo"] = np.ascontiguousarray(np.concatenate([hi, lo], axis=0))
        else:
            m["xT"] = xT
        m.update(w_np)
        m.update(b_np)
        maps.append(m)
    return maps


def kernel(**inputs):
    x = np.asarray(inputs["x"])
    in_dtype = x.dtype if x.dtype in (np.float32, np.float64) else np.float64
    cdtype = np.complex64 if in_dtype == np.float32 else np.complex128

    ews = [np.asarray(inputs[f"enc_w{i}"]) for i in range(4)]
    ebs = [np.asarray(inputs[f"enc_b{i}"]) for i in range(4)]
    dws = [np.asarray(inputs[f"dec_w{i}"]) for i in range(4)]
    dbs = [np.asarray(inputs[f"dec_b{i}"]) for i in range(4)]

    # ---------------- device: y = enc(x), x_ae = dec(y) ----------------
    scheme = SCHEME
    zero_bias = all(not np.any(b) for b in ebs + dbs)
    nc = _get_nc(scheme, zero_bias)
    in_maps = _in_maps_for(scheme, x.astype(np.float32, copy=False), ews + dws,
                           ebs + dbs, zero_bias=zero_bias)
    # x64 mode makes the bass2jax compile pathological (~15x slower); the
    # device program is pure float32, so pin x64 off around the launch.
    import jax
    x64_prev = bool(jax.config.jax_enable_x64)
    try:
        jax.config.update("jax_enable_x64", False)
        res = run_bass_kernel_spmd(nc, in_maps, core_ids=list(range(N_CORES)))
    finally:
        jax.config.update("jax_enable_x64", x64_prev)

    y_dev = np.empty((B, T, L), dtype=np.float32)
    xae_dev = np.empty((B, T, D), dtype=np.float32)
    for c in range(N_CORES):
        y_dev[c * B_SH:(c + 1) * B_SH] = res.results[c]["yT"].T.reshape(B_SH, T, L)
        xae_dev[c * B_SH:(c + 1) * B_SH] = res.results[c]["xaeT"].T.reshape(B_SH, T, D)

    # ------- host: bit-exact reference EDMD chain in input precision -------
    (_y_host, k_efuns, k_evals, k_modes, y_adv, x_adv) = _host_chain(
        x, ews, ebs, dws, dbs, in_dtype)
    y_adv_real = np.real(y_adv)
    y_adv_imag = np.imag(y_adv)

    return (
        y_dev.astype(in_dtype),
        xae_dev.astype(in_dtype),
        x_adv.astype(in_dtype),
        y_adv_real.astype(in_dtype),
        y_adv_imag.astype(in_dtype),
        k_evals.astype(cdtype),
        k_efuns.astype(cdtype),
        k_modes.astype(cdtype),
    )
